# revision 1
# baseline (speedup 1.0000x reference)
"""Causal self-attention (B=4, T=2048, C=1024, H=16) on 8 trn2 NeuronCores.

Sharding: core c = (batch b = c//2, head-group g = c%2). Each core computes
the full attention for batch b and heads 8g..8g+7 (column-parallel qkv,
row-parallel proj), producing a partial [T, C] output; the host sums the two
partials per batch.

Per-core device kernel (Bass/Tile, SPMD same program on all 8 cores):
  qT/kT  [512, T] = (wq|wk).T @ x.T        (bf16 matmuls, fp32 psum)
  v      [T, 8, 65]  (natural layout, ones column appended per head)
  S^T    [tk 128, tq 512] blocks = kT.T-slices @ qT-slices (2 heads row-packed)
  P^T    = exp((S^T + causal_mask)/8)      (ScalarE, psum->sbuf bf16)
  y/l    = [v|1].T @ P^T  accumulated over tk  -> [65, tq] psum per head
  yT_n   = yT * replicate(1/l)             (recip on DVE, replicate via K=2 matmul)
  out    = yT_n.T @ wo  -> [T, C] fp32 partial
"""

import os
import sys

import numpy as np

import concourse.bacc as bacc
import concourse.bass as bass
import concourse.mybir as mybir
import concourse.tile as tile
from concourse.bass_utils import run_bass_kernel_spmd

try:
    import ml_dtypes

    BF16 = np.dtype(ml_dtypes.bfloat16)
except ImportError:  # pragma: no cover
    BF16 = np.dtype("bfloat16")

B, T, C = 4, 2048, 1024
N_HEAD = 16
D = 64  # head dim
H_LOC = 8  # heads per core
DL = H_LOC * D  # 512, local d width per core
CK = C // 128  # 8 contraction chunks
DT = mybir.dt.bfloat16
F32 = mybir.dt.float32
F32R = mybir.dt.float32r
NEG = -1.0e9


def build_program(t_len=T, enable_asserts=False, debug_dump=False):
    """Build the SPMD per-core program. Returns the compiled Bacc object."""
    NJ = t_len // 512  # tq chunks
    NTT = t_len // 128  # 128-wide t tiles
    MD = DL // 128  # 4 d-chunks of qT/kT/yT

    nc = bacc.Bacc(
        "TRN2",
        target_bir_lowering=False,
        debug=False,
        enable_asserts=enable_asserts,
        num_devices=8,
    )

    xT_d = nc.dram_tensor("xT", [C, t_len], DT, kind="ExternalInput").ap()
    wq_d = nc.dram_tensor("wq", [C, DL], DT, kind="ExternalInput").ap()
    wk_d = nc.dram_tensor("wk", [C, DL], DT, kind="ExternalInput").ap()
    wv_d = nc.dram_tensor("wv", [C, DL], DT, kind="ExternalInput").ap()
    wo_d = nc.dram_tensor("wo", [DL, C], DT, kind="ExternalInput").ap()
    mask_d = nc.dram_tensor("mask", [128, 128], F32, kind="ExternalInput").ap()
    out_d = nc.dram_tensor("out", [t_len, C], F32, kind="ExternalOutput").ap()
    dbg = {}
    if debug_dump:
        dbg["dqt"] = nc.dram_tensor("dqt", [128, MD, t_len], DT, kind="ExternalOutput").ap()
        dbg["dkt"] = nc.dram_tensor("dkt", [128, MD, t_len], DT, kind="ExternalOutput").ap()
        dbg["dv"] = nc.dram_tensor("dv", [128, NTT, H_LOC, D + 1], DT, kind="ExternalOutput").ap()
        dbg["dyt"] = nc.dram_tensor("dyt", [128, MD, t_len], DT, kind="ExternalOutput").ap()
        dbg["dpt0"] = nc.dram_tensor("dpt0", [128, 2, 512], DT, kind="ExternalOutput").ap()
        dbg["dpt1"] = nc.dram_tensor("dpt1", [128, 2, 512], DT, kind="ExternalOutput").ap()
        dbg["dyuA"] = nc.dram_tensor("dyuA", [D + 1, 512], DT, kind="ExternalOutput").ap()
        dbg["dyuB"] = nc.dram_tensor("dyuB", [D + 1, 512], DT, kind="ExternalOutput").ap()
        dbg["drliA"] = nc.dram_tensor("drliA", [64, 512], F32, kind="ExternalOutput").ap()

    with tile.TileContext(nc) as tc:
        with (
            tc.tile_pool(name="consts", bufs=1) as cpool,
            tc.tile_pool(name="ptp", bufs=4) as pt_pool,
            tc.tile_pool(name="yup", bufs=3) as yu_pool,
            tc.tile_pool(name="rlp", bufs=3) as rl_pool,
            tc.tile_pool(name="outp", bufs=3) as out_pool,
            tc.tile_pool(name="psum", bufs=1, space="PSUM") as psum,
        ):
            # ---- persistent SBUF tensors ----
            xt_t = cpool.tile([128, CK, t_len], DT, name="xt")
            wq_t = cpool.tile([128, CK, DL], DT, name="wqt")
            wk_t = cpool.tile([128, CK, DL], DT, name="wkt")
            wv_t = cpool.tile([128, CK, DL], DT, name="wvt")
            wo_t = cpool.tile([128, MD, C], DT, name="wot")
            qt_t = cpool.tile([128, MD, t_len], DT, name="qtt")
            kt_t = cpool.tile([128, MD, t_len], DT, name="ktt")
            v_t = cpool.tile([128, NTT, H_LOC, D + 1], DT, name="vt")
            yt_t = cpool.tile([128, MD, t_len], DT, name="ytt")
            mask_t = cpool.tile([128, 2, 128], F32, name="maskt")
            ones1_t = cpool.tile([D + 1, 64], DT, name="ones1t")

            # ---- input DMAs (per-chunk so compute can start early) ----
            xT_v = xT_d.rearrange("(n p) t -> n p t", p=128)
            wq_v = wq_d.rearrange("(n p) d -> n p d", p=128)
            wk_v = wk_d.rearrange("(n p) d -> n p d", p=128)
            wv_v = wv_d.rearrange("(n p) d -> n p d", p=128)
            wo_v = wo_d.rearrange("(n p) c -> n p c", p=128)
            for k in range(CK):
                nc.sync.dma_start(out=xt_t[:, k, :], in_=xT_v[k])
                nc.sync.dma_start(out=wq_t[:, k, :], in_=wq_v[k])
                nc.sync.dma_start(out=wk_t[:, k, :], in_=wk_v[k])
                nc.sync.dma_start(out=wv_t[:, k, :], in_=wv_v[k])
            for m in range(MD):
                nc.sync.dma_start(out=wo_t[:, m, :], in_=wo_v[m])
            nc.sync.dma_start(out=mask_t[:, 0, :], in_=mask_d)
            nc.sync.dma_start(out=mask_t[:, 1, :], in_=mask_d)
            # ones column (index 64) for the l (softmax denominator) rows
            nc.vector.memset(v_t[:, :, :, D : D + 1], 1.0)
            nc.vector.memset(ones1_t[:, :], 1.0)

            # ---- qkv ----
            def qk_proj(w_t, dst_t, m, j):
                ps = psum.tile([128, 512], F32, name="qkvps", bufs=2)
                for k in range(CK):
                    nc.tensor.matmul(
                        ps[:, :],
                        lhsT=w_t[:, k, 128 * m : 128 * (m + 1)],
                        rhs=xt_t[:, k, 512 * j : 512 * (j + 1)],
                        start=(k == 0),
                        stop=(k == CK - 1),
                    )
                nc.vector.tensor_copy(dst_t[:, m, 512 * j : 512 * (j + 1)], ps[:, :])

            def v_stage():
                for ti in range(NTT):
                    ps = psum.tile([128, 512], F32, name="qkvps", bufs=2)
                    for k in range(CK):
                        nc.tensor.matmul(
                            ps[:, :],
                            lhsT=xt_t[:, k, 128 * ti : 128 * (ti + 1)],
                            rhs=wv_t[:, k, :],
                            start=(k == 0),
                            stop=(k == CK - 1),
                        )
                    nc.vector.tensor_copy(
                        v_t[:, ti, :, 0:D],
                        ps[:, :].rearrange("p (h d) -> p h d", h=H_LOC),
                    )

            def attn_stage(hp):
                for j in range(NJ):
                    tq0 = 512 * j
                    nblk = 4 * j + 4  # causal: tk blocks 0 .. 4j+3
                    accA = psum.tile([D + 1, 512], F32, name="acc", bufs=2)
                    accB = psum.tile([D + 1, 512], F32, name="acc", bufs=2)
                    pend = []  # software pipeline: AV for block i-1 after S of i

                    def flush_av():
                        for mm in pend:
                            nc.tensor.matmul(**mm)
                        pend.clear()

                    for i in range(nblk):
                        tk = slice(128 * i, 128 * (i + 1))
                        diag = i - 4 * j
                        lo = 128 * diag if diag >= 0 else 0
                        tqs = slice(tq0 + lo, tq0 + 512)
                        sps = psum.tile([128, 2, 512], F32, name="sps", bufs=2)
                        for h2, lohi in ((0, slice(0, 64)), (1, slice(64, 128))):
                            nc.tensor.matmul(
                                sps[:, h2, lo:],
                                lhsT=kt_t[lohi, hp, tk],
                                rhs=qt_t[lohi, hp, tqs],
                                start=True,
                                stop=True,
                            )
                        if diag >= 0:  # block crosses the causal diagonal
                            dg = slice(lo, lo + 128)
                            nc.vector.tensor_add(
                                sps[:, :, dg], sps[:, :, dg], mask_t[:, :, :]
                            )
                        pt = pt_pool.tile([128, 2, 512], DT, name="pt")
                        nc.scalar.activation(
                            pt[:, :, lo:],
                            sps[:, :, lo:],
                            mybir.ActivationFunctionType.Exp,
                            scale=0.125,
                        )
                        if debug_dump and hp == 0 and j == 0 and i in (0, 1):
                            nc.sync.dma_start(
                                out=dbg[f"dpt{i}"][:, :, lo:], in_=pt[:, :, lo:]
                            )
                        flush_av()
                        for h2, acc in ((0, accA), (1, accB)):
                            pend.append(
                                dict(
                                    out=acc[:, lo:],
                                    lhsT=v_t[:, i, 2 * hp + h2, :],
                                    rhs=pt[:, h2, lo:],
                                    start=(i == 0),
                                    stop=(i == nblk - 1),
                                )
                            )
                    flush_av()

                    # normalization: replicate l via K=1 matmul, recip, multiply
                    tq = slice(tq0, tq0 + 512)
                    yuA = yu_pool.tile([D + 1, 512], DT, name="yuA")
                    yuB = yu_pool.tile([D + 1, 512], DT, name="yuB")
                    nc.vector.tensor_copy(yuA[:, :], accA[:, :])
                    nc.vector.tensor_copy(yuB[:, :], accB[:, :])
                    repA = psum.tile([64, 512], F32, name="sps", bufs=2)
                    repB = psum.tile([64, 512], F32, name="sps", bufs=2)
                    nc.tensor.matmul(
                        repA[:, :],
                        lhsT=ones1_t[D : D + 1, :],
                        rhs=yuA[D : D + 1, :],
                        start=True,
                        stop=True,
                        tile_position=(64, 0),
                    )
                    nc.tensor.matmul(
                        repB[:, :],
                        lhsT=ones1_t[D : D + 1, :],
                        rhs=yuB[D : D + 1, :],
                        start=True,
                        stop=True,
                        tile_position=(64, 0),
                    )
                    rliA = rl_pool.tile([64, 512], F32, name="rliA")
                    rliB = rl_pool.tile([64, 512], F32, name="rliB")
                    nc.vector.reciprocal_approx_fast(rliA[:, :], repA[:, :])
                    nc.vector.reciprocal_approx_fast(rliB[:, :], repB[:, :])
                    if debug_dump and hp == 0 and j == 0:
                        nc.sync.dma_start(out=dbg["dyuA"], in_=yuA[:, :])
                        nc.sync.dma_start(out=dbg["dyuB"], in_=yuB[:, :])
                        nc.sync.dma_start(out=dbg["drliA"], in_=rliA[:, :])
                    nc.vector.tensor_mul(yt_t[0:64, hp, tq], yuA[0:D, :], rliA[:, :])
                    nc.vector.tensor_mul(yt_t[64:128, hp, tq], yuB[0:D, :], rliB[:, :])

            # interleave: v, then per d-chunk m: qk[m] followed by attention on
            # head pair m (keeps ScalarE busy from early on)
            v_stage()
            for m in range(MD):
                for j in range(NJ):
                    qk_proj(wq_t, qt_t, m, j)
                    qk_proj(wk_t, kt_t, m, j)
                attn_stage(m)

            # ---- proj ----
            for ti in range(NTT):
                tt = slice(128 * ti, 128 * (ti + 1))
                ot = out_pool.tile([128, C], F32, name="ot")
                for ci in range(2):
                    cs = slice(512 * ci, 512 * (ci + 1))
                    ps = psum.tile([128, 512], F32, name="qkvps", bufs=2)
                    for hp in range(MD):
                        nc.tensor.matmul(
                            ps[:, :],
                            lhsT=yt_t[:, hp, tt],
                            rhs=wo_t[:, hp, cs],
                            start=(hp == 0),
                            stop=(hp == MD - 1),
                        )
                    nc.vector.tensor_copy(ot[:, cs], ps[:, :])
                nc.sync.dma_start(out=out_d[tt, :], in_=ot[:, :])

            if debug_dump:
                nc.sync.dma_start(out=dbg["dqt"], in_=qt_t[:, :, :])
                nc.sync.dma_start(out=dbg["dkt"], in_=kt_t[:, :, :])
                nc.sync.dma_start(out=dbg["dv"], in_=v_t[:, :, :, :])
                nc.sync.dma_start(out=dbg["dyt"], in_=yt_t[:, :, :])

    nc.compile()
    return nc


def make_host_inputs(x, w_qkv, w_proj, t_len=T):
    """Shard full inputs into the 8 per-core input dicts."""
    mask = np.where(
        np.arange(128)[None, :] >= np.arange(128)[:, None], 0.0, NEG
    ).astype(np.float32)

    in_maps = []
    for c in range(8):
        b, g = c // 2, c % 2
        xT = np.ascontiguousarray(x[b][:t_len].T).astype(BF16)
        wq = w_qkv[:, 512 * g : 512 * (g + 1)].astype(BF16)
        wk = w_qkv[:, C + 512 * g : C + 512 * (g + 1)].astype(BF16)
        wv = w_qkv[:, 2 * C + 512 * g : 2 * C + 512 * (g + 1)].astype(BF16)
        wo = np.ascontiguousarray(w_proj[512 * g : 512 * (g + 1), :]).astype(BF16)
        in_maps.append(dict(xT=xT, wq=wq, wk=wk, wv=wv, wo=wo, mask=mask))
    return in_maps


_CACHE = {}


def _get_program():
    if "nc" not in _CACHE:
        _CACHE["nc"] = build_program()
    return _CACHE["nc"]


def kernel(x, w_qkv, w_proj, _trace=False, _trace_kwargs=None):
    x = np.asarray(x, np.float32)
    w_qkv = np.asarray(w_qkv, np.float32)
    w_proj = np.asarray(w_proj, np.float32)
    nc = _get_program()
    in_maps = make_host_inputs(x, w_qkv, w_proj)
    kw = {}
    if _trace:
        kw = dict(trace=True, **(_trace_kwargs or {}))
    res = run_bass_kernel_spmd(nc, in_maps, core_ids=list(range(8)), **kw)
    out = np.empty((B, T, C), np.float32)
    for b in range(B):
        out[b] = res.results[2 * b]["out"] + res.results[2 * b + 1]["out"]
    if _trace:
        return out, res
    return out



# revision 12
# speedup vs baseline: 1.1441x; 1.1441x over previous
"""Causal self-attention (B=4, T=2048, C=1024, H=16) on 8 trn2 NeuronCores.

Sharding: core c = (batch b = c//2, head-group g = c%2). Each core computes
the full attention for batch b and heads 8g..8g+7 (column-parallel qkv,
row-parallel proj), producing a partial [T, C] output (bf16); the host sums
the two partials per batch in fp32.

Per-core device kernel (Bass/Tile, SPMD same program on all 8 cores):
  warmup  ~20 dummy matmuls so the PE HAM clock-gate is warm before real work
  qT/kT  [512, T] = (wq|wk).T @ x.T        (bf16 matmuls, fp32 psum)
  v      [T, 8, 65]  (natural layout, ones column appended per head)
  S^T    [tk 128, tq 512] blocks = kT.T-slices @ qT-slices (2 heads row-packed)
  P^T    = exp(S^T/8) then 0/1-triangle multiply on the diagonal tile (DVE)
  y/l    = [v|1].T @ P^T  accumulated over tk  -> [65, tq] psum per head
  1/l    via DVE reciprocal_approx_fast + GpSimd partition_broadcast
  yT_n   = yT * bcast(1/l)                 (GpSimd)
  out    = yT_n.T @ wo -> [T, C] bf16 partial

The PE instruction stream is explicitly interleaved: during attention stage
m the projection matmuls of stage m+1 (and the v/proj streams) are emitted
~2 per attention block so the PE never idles on ScalarE's exp.
"""

import numpy as np

import concourse.bacc as bacc
import concourse.bass as bass
import concourse.library_config as library_config
import concourse.mybir as mybir
import concourse.tile as tile
from concourse.bass_utils import run_bass_kernel_spmd

try:
    import ml_dtypes

    BF16 = np.dtype(ml_dtypes.bfloat16)
except ImportError:  # pragma: no cover
    BF16 = np.dtype("bfloat16")

B, T, C = 4, 2048, 1024
N_HEAD = 16
D = 64  # head dim
H_LOC = 8  # heads per core
DL = H_LOC * D  # 512, local d width per core
CK = C // 128  # 8 contraction chunks
DT = mybir.dt.bfloat16
F32 = mybir.dt.float32
N_WARM = 20  # warmup matmuls to open the HAM clock gate during input DMA


def build_program(t_len=T, enable_asserts=False, debug_dump=False):
    """Build the SPMD per-core program. Returns the compiled Bacc object."""
    NJ = t_len // 512  # tq chunks
    NTT = t_len // 128  # 128-wide t tiles
    MD = DL // 128  # 4 d-chunks of qT/kT/yT

    nc = bacc.Bacc(
        "TRN2",
        target_bir_lowering=False,
        debug=False,
        enable_asserts=enable_asserts,
        num_devices=8,
    )

    xT_d = nc.dram_tensor("xT", [C, t_len], DT, kind="ExternalInput").ap()
    wq_d = nc.dram_tensor("wq", [C, DL], DT, kind="ExternalInput").ap()
    wk_d = nc.dram_tensor("wk", [C, DL], DT, kind="ExternalInput").ap()
    wv_d = nc.dram_tensor("wv", [C, DL], DT, kind="ExternalInput").ap()
    wo_d = nc.dram_tensor("wo", [DL, C], DT, kind="ExternalInput").ap()
    tri_d = nc.dram_tensor("tri", [128, 128], DT, kind="ExternalInput").ap()
    out_d = nc.dram_tensor("out", [t_len, C], DT, kind="ExternalOutput").ap()
    dbg = {}
    if debug_dump:
        dbg["drl1"] = nc.dram_tensor("drl1", [1, 512], F32, kind="ExternalOutput").ap()
        dbg["drl1b"] = nc.dram_tensor("drl1b", [1, 512], DT, kind="ExternalOutput").ap()
        dbg["drli"] = nc.dram_tensor("drli", [64, 512], DT, kind="ExternalOutput").ap()
        dbg["dyu"] = nc.dram_tensor("dyu", [64, 512], DT, kind="ExternalOutput").ap()
        dbg["dacc"] = nc.dram_tensor("dacc", [D + 1, 512], F32, kind="ExternalOutput").ap()

    with tile.TileContext(nc) as tc:
        with (
            tc.tile_pool(name="consts", bufs=1) as cpool,
            tc.tile_pool(name="ptp", bufs=4) as pt_pool,
            tc.tile_pool(name="yup", bufs=3) as yu_pool,
            tc.tile_pool(name="rlp", bufs=3) as rl_pool,
            tc.tile_pool(name="outp", bufs=3) as out_pool,
            tc.tile_pool(name="psum", bufs=1, space="PSUM") as psum,
        ):
            # ---- persistent SBUF tensors ----
            xt_t = cpool.tile([128, CK, t_len], DT, name="xt")
            wq_t = cpool.tile([128, CK, DL], DT, name="wqt")
            wk_t = cpool.tile([128, CK, DL], DT, name="wkt")
            wv_t = cpool.tile([128, CK, DL], DT, name="wvt")
            wo_t = cpool.tile([128, MD, C], DT, name="wot")
            qt_t = cpool.tile([128, MD, t_len], DT, name="qtt")
            kt_t = cpool.tile([128, MD, t_len], DT, name="ktt")
            v_t = cpool.tile([128, NTT, H_LOC, D + 1], DT, name="vt")
            yt_t = cpool.tile([128, MD, t_len], DT, name="ytt")
            tri_t = cpool.tile([128, 2, 128], DT, name="trit")
            warm_t = cpool.tile([128, 512], DT, name="warmt")

            # ---- input DMAs, ordered by when compute needs them ----
            xT_v = xT_d.rearrange("(k p) t -> p k t", p=128)
            wq_v = wq_d.rearrange("(k p) d -> p k d", p=128)
            wk_v = wk_d.rearrange("(k p) d -> p k d", p=128)
            wv_v = wv_d.rearrange("(k p) d -> p k d", p=128)
            wo_v = wo_d.rearrange("(m p) c -> p m c", p=128)
            NQ = max(1, t_len // 512)  # x arrives in t-quarters
            nc.sync.dma_start(out=tri_t[:, 0, :], in_=tri_d)
            nc.sync.dma_start(out=tri_t[:, 1, :], in_=tri_d)
            nc.sync.dma_start(out=wq_t[:, :, :], in_=wq_v)
            nc.sync.dma_start(out=xt_t[:, :, 0:512], in_=xT_v[:, :, 0:512])
            nc.sync.dma_start(out=wk_t[:, :, :], in_=wk_v)
            nc.sync.dma_start(out=wv_t[:, :, :], in_=wv_v)
            for q in range(1, NQ):
                qs = slice(512 * q, 512 * (q + 1))
                nc.sync.dma_start(out=xt_t[:, :, qs], in_=xT_v[:, :, qs])
            nc.sync.dma_start(out=wo_t[:, :, :], in_=wo_v)

            # ones column (index 64) for the l (softmax denominator) rows
            nc.vector.memset(v_t[:, :, :, D : D + 1], 1.0)
            nc.vector.memset(warm_t[:, :], 0.25)
            # partition_broadcast ucode lives in the `proxy` library, not the
            # default-resident `standard` one — load it before any gpsimd op
            nc.gpsimd.load_library(library_config.proxy)

            # ---- warmup: keep PE busy while inputs stream in ----
            for _ in range(N_WARM):
                wps = psum.tile([128, 512], F32, name="qkvps", bufs=2)
                nc.tensor.matmul(
                    wps[:, :],
                    lhsT=warm_t[:, 0:128],
                    rhs=warm_t[:, :],
                    start=True,
                    stop=True,
                )

            # ---- projection step generators (one yield per matmul) ----
            def qk_steps(m, jlist):
                for w_t, dst_t in ((wq_t, qt_t), (wk_t, kt_t)):
                    for j in jlist:
                        ps = psum.tile([128, 512], F32, name="qkvps", bufs=2)
                        for k in range(CK):
                            nc.tensor.matmul(
                                ps[:, :],
                                lhsT=w_t[:, k, 128 * m : 128 * (m + 1)],
                                rhs=xt_t[:, k, 512 * j : 512 * (j + 1)],
                                start=(k == 0),
                                stop=(k == CK - 1),
                            )
                            if k < CK - 1:
                                yield
                        nc.vector.tensor_copy(
                            dst_t[:, m, 512 * j : 512 * (j + 1)], ps[:, :]
                        )
                        yield

            def v_steps(t0, t1):
                for ti in range(t0, t1):
                    ps = psum.tile([128, 512], F32, name="qkvps", bufs=2)
                    for k in range(CK):
                        nc.tensor.matmul(
                            ps[:, :],
                            lhsT=xt_t[:, k, 128 * ti : 128 * (ti + 1)],
                            rhs=wv_t[:, k, :],
                            start=(k == 0),
                            stop=(k == CK - 1),
                        )
                        if k < CK - 1:
                            yield
                    nc.vector.tensor_copy(
                        v_t[:, ti, :, 0:D],
                        ps[:, :].rearrange("p (h d) -> p h d", h=H_LOC),
                    )
                    yield

            def proj_steps():
                for ti in range(NTT):
                    tt = slice(128 * ti, 128 * (ti + 1))
                    ot = out_pool.tile([128, C], DT, name="ot")
                    for ci in range(2):
                        cs = slice(512 * ci, 512 * (ci + 1))
                        ps = psum.tile([128, 512], F32, name="qkvps", bufs=2)
                        for hp in range(MD):
                            nc.tensor.matmul(
                                ps[:, :],
                                lhsT=yt_t[:, hp, tt],
                                rhs=wo_t[:, hp, cs],
                                start=(hp == 0),
                                stop=(hp == MD - 1),
                            )
                            if hp < MD - 1:
                                yield
                        nc.vector.tensor_copy(ot[:, cs], ps[:, :])
                        if ci == 0:
                            yield
                    nc.sync.dma_start(out=out_d[tt, :], in_=ot[:, :])
                    yield

            # ---- filler stream: consumed 2 steps per attention block ----
            def filler_gen():
                yield from v_steps(8, 12)
                yield "v11"
                yield from v_steps(12, 16)
                yield "v15"
                for m in range(1, MD):
                    yield from qk_steps(m, range(NJ))
                    yield f"qk{m}"

            fill = {"it": filler_gen(), "seen": set(), "done": False}

            def consume(n):
                if fill["done"]:
                    return
                got = 0
                while got < n:
                    try:
                        item = next(fill["it"])
                    except StopIteration:
                        fill["done"] = True
                        return
                    if isinstance(item, str):
                        fill["seen"].add(item)
                    else:
                        got += 1

            def drain(tag):
                if fill["done"] or tag in fill["seen"]:
                    return
                while True:
                    try:
                        item = next(fill["it"])
                    except StopIteration:
                        fill["done"] = True
                        return
                    if isinstance(item, str):
                        fill["seen"].add(item)
                        if item == tag:
                            return

            proj = {"it": None, "done": True}

            def consume_proj(n):
                if proj["done"]:
                    return
                got = 0
                while got < n:
                    try:
                        next(proj["it"])
                        got += 1
                    except StopIteration:
                        proj["done"] = True
                        return

            # ---- attention stage for head-pair hp ----
            def attn_stage(hp):
                for j in range(NJ):
                    if hp == 0 and j == 2:
                        drain("v11")
                    if hp == 0 and j == 3:
                        drain("v15")
                    tq0 = 512 * j
                    nblk = 4 * j + 4  # causal: tk blocks 0 .. 4j+3
                    accA = psum.tile([D + 1, 512], F32, name="acc", bufs=2)
                    accB = psum.tile([D + 1, 512], F32, name="acc", bufs=2)
                    pend = []  # software pipeline: AV for block i-1 after S of i

                    def flush_av():
                        for mm in pend:
                            nc.tensor.matmul(**mm)
                        pend.clear()

                    for i in range(nblk):
                        tk = slice(128 * i, 128 * (i + 1))
                        diag = i - 4 * j
                        lo = 128 * diag if diag >= 0 else 0
                        tqs = slice(tq0 + lo, tq0 + 512)
                        sps = psum.tile([128, 2, 512], F32, name="sps", bufs=2)
                        for h2, lohi in ((0, slice(0, 64)), (1, slice(64, 128))):
                            nc.tensor.matmul(
                                sps[:, h2, lo:],
                                lhsT=kt_t[lohi, hp, tk],
                                rhs=qt_t[lohi, hp, tqs],
                                start=True,
                                stop=True,
                            )
                        pt = pt_pool.tile([128, 2, 512], DT, name="pt")
                        nc.scalar.activation(
                            pt[:, :, lo:],
                            sps[:, :, lo:],
                            mybir.ActivationFunctionType.Exp,
                            scale=0.125,
                        )
                        if diag >= 0:  # zero the above-diagonal triangle
                            dg = slice(lo, lo + 128)
                            nc.vector.tensor_mul(
                                pt[:, :, dg], pt[:, :, dg], tri_t[:, :, :]
                            )
                        if hp < MD - 1:
                            consume(2)
                        elif j >= 1:
                            # proj tiles for tq chunk j-1 are fully emitted by
                            # now; consuming earlier would emit proj matmuls
                            # ahead of the yt writes they need (PE deadlock)
                            consume_proj(2)
                        flush_av()
                        for h2, acc in ((0, accA), (1, accB)):
                            pend.append(
                                dict(
                                    out=acc[:, lo:],
                                    lhsT=v_t[:, i, 2 * hp + h2, :],
                                    rhs=pt[:, h2, lo:],
                                    start=(i == 0),
                                    stop=(i == nblk - 1),
                                )
                            )
                    flush_av()

                    # normalization: broadcast l (GpSimd), then 1/l on the
                    # 64-partition broadcast (reciprocal_approx_fast gives
                    # wrong results on HW for single-partition inputs)
                    tq = slice(tq0, tq0 + 512)
                    for h2, acc in ((0, accA), (1, accB)):
                        lb = rl_pool.tile([1, 512], F32, name="lb")
                        nc.vector.tensor_copy(lb[:, :], acc[D : D + 1, :])
                        lbc = rl_pool.tile([64, 512], F32, name="lbc")
                        nc.gpsimd.partition_broadcast(lbc[:, :], lb[:, :])
                        rli = rl_pool.tile([64, 512], F32, name="rli")
                        nc.vector.reciprocal_approx_fast(rli[:, :], lbc[:, :])
                        yu = yu_pool.tile([64, 512], DT, name="yu")
                        nc.vector.tensor_copy(yu[:, :], acc[0:D, :])
                        nc.vector.tensor_mul(
                            yt_t[64 * h2 : 64 * (h2 + 1), hp, tq],
                            yu[:, :],
                            rli[:, :],
                        )

            # ---- main pipeline ----
            # prologue: q/k for stage 0 and v tiles 0-7, ordered by DMA arrival
            for _ in qk_steps(0, [0]):
                pass
            for _ in v_steps(0, 4):
                pass
            for _ in qk_steps(0, [1]):
                pass
            for _ in v_steps(4, 8):
                pass
            for _ in qk_steps(0, [2, 3]):
                pass

            for hp in range(MD):
                if hp >= 1:
                    drain(f"qk{hp}")
                if hp == MD - 1:
                    proj.update(it=proj_steps(), done=False)
                attn_stage(hp)

            consume_proj(10**9)  # finish remaining projection tiles

    nc.compile()
    return nc


def make_host_inputs(x, w_qkv, w_proj, t_len=T):
    """Shard full inputs into the 8 per-core input dicts."""
    tri = np.where(
        np.arange(128)[None, :] >= np.arange(128)[:, None], 1.0, 0.0
    ).astype(BF16)

    in_maps = []
    for c in range(8):
        b, g = c // 2, c % 2
        xT = np.ascontiguousarray(x[b][:t_len].T).astype(BF16)
        wq = w_qkv[:, 512 * g : 512 * (g + 1)].astype(BF16)
        wk = w_qkv[:, C + 512 * g : C + 512 * (g + 1)].astype(BF16)
        wv = w_qkv[:, 2 * C + 512 * g : 2 * C + 512 * (g + 1)].astype(BF16)
        wo = np.ascontiguousarray(w_proj[512 * g : 512 * (g + 1), :]).astype(BF16)
        in_maps.append(dict(xT=xT, wq=wq, wk=wk, wv=wv, wo=wo, tri=tri))
    return in_maps


_CACHE = {}


def _get_program():
    if "nc" not in _CACHE:
        _CACHE["nc"] = build_program()
    return _CACHE["nc"]


def kernel(x, w_qkv, w_proj, _trace=False, _trace_kwargs=None):
    x = np.asarray(x, np.float32)
    w_qkv = np.asarray(w_qkv, np.float32)
    w_proj = np.asarray(w_proj, np.float32)
    nc = _get_program()
    in_maps = make_host_inputs(x, w_qkv, w_proj)
    kw = {}
    if _trace:
        kw = dict(trace=True, **(_trace_kwargs or {}))
    res = run_bass_kernel_spmd(nc, in_maps, core_ids=list(range(8)), **kw)
    out = np.empty((B, T, C), np.float32)
    for b in range(B):
        out[b] = res.results[2 * b]["out"].astype(np.float32) + res.results[
            2 * b + 1
        ]["out"].astype(np.float32)
    if _trace:
        return out, res
    return out


# revision 13
# speedup vs baseline: 1.1521x; 1.0071x over previous
"""Causal self-attention (B=4, T=2048, C=1024, H=16) on 8 trn2 NeuronCores.

Sharding: core c = (batch b = c//2, head-group g = c%2). Each core computes
the full attention for batch b and heads 8g..8g+7 (column-parallel qkv,
row-parallel proj), producing a partial [T, C] output (bf16); the host sums
the two partials per batch in fp32.

Per-core device kernel (Bass/Tile, SPMD same program on all 8 cores):
  warmup  dummy matmuls so the PE HAM clock-gate is warm before real work
  qT/kT  [512, T] = (wq|wk).T @ x.T        (bf16 matmuls, fp32 psum)
  v      [T, 8, 65]  (natural layout, ones column appended per head)
  S^T    [tk 128, tq 512] blocks = kT.T-slices @ qT-slices (2 heads row-packed)
  P^T    = exp(S^T/8) then 0/1-triangle multiply on the diagonal tile (DVE)
  y/l    = [v|1].T @ P^T  accumulated over tk  -> [65, tq] psum per head
  1/l    via GpSimd partition_broadcast of l + DVE reciprocal_approx_fast
         (the reciprocal runs on the 64-partition broadcast: it gives wrong
         results on HW for single-partition inputs)
  yT_n   = yT * (1/l)                      (DVE)
  out    = yT_n.T @ wo -> [T, C] bf16 partial

All inputs are pre-arranged on the host into the exact SBUF layout
([partition, chunk, free], contiguous) so every input DMA is a single
large-run transfer. The PE instruction stream is explicitly interleaved:
during attention stage m the projection matmuls of stage m+1 (and the v /
proj streams) are emitted ~2 per attention block so the PE never idles on
ScalarE's exp.
"""

import numpy as np

import concourse.bacc as bacc
import concourse.bass as bass
import concourse.library_config as library_config
import concourse.mybir as mybir
import concourse.tile as tile
from concourse.bass_utils import run_bass_kernel_spmd

try:
    import ml_dtypes

    BF16 = np.dtype(ml_dtypes.bfloat16)
except ImportError:  # pragma: no cover
    BF16 = np.dtype("bfloat16")

B, T, C = 4, 2048, 1024
N_HEAD = 16
D = 64  # head dim
H_LOC = 8  # heads per core
DL = H_LOC * D  # 512, local d width per core
CK = C // 128  # 8 contraction chunks
DT = mybir.dt.bfloat16
F32 = mybir.dt.float32
N_WARM = 26  # warmup matmuls to open the HAM clock gate during input DMA
N_WARM2 = 8  # extra warmups bridging the first DMA-paced prologue waits


def build_program(t_len=T, enable_asserts=False):
    """Build the SPMD per-core program. Returns the compiled Bacc object."""
    NJ = t_len // 512  # tq chunks
    NTT = t_len // 128  # 128-wide t tiles
    MD = DL // 128  # 4 d-chunks of qT/kT/yT

    nc = bacc.Bacc(
        "TRN2",
        target_bir_lowering=False,
        debug=False,
        enable_asserts=enable_asserts,
        num_devices=8,
    )

    x_d = [
        nc.dram_tensor(f"x{q}", [128, CK, 512], DT, kind="ExternalInput").ap()
        for q in range(NJ)
    ]
    wq_d = nc.dram_tensor("wq", [128, CK, DL], DT, kind="ExternalInput").ap()
    wk_d = nc.dram_tensor("wk", [128, CK, DL], DT, kind="ExternalInput").ap()
    wv_d = nc.dram_tensor("wv", [128, CK, DL], DT, kind="ExternalInput").ap()
    wo_d = nc.dram_tensor("wo", [128, MD, C], DT, kind="ExternalInput").ap()
    tri_d = nc.dram_tensor("tri", [128, 128], DT, kind="ExternalInput").ap()
    out_d = nc.dram_tensor("out", [t_len, C], DT, kind="ExternalOutput").ap()

    with tile.TileContext(nc) as tc:
        with (
            tc.tile_pool(name="consts", bufs=1) as cpool,
            tc.tile_pool(name="ptp", bufs=4) as pt_pool,
            tc.tile_pool(name="yup", bufs=3) as yu_pool,
            tc.tile_pool(name="rlp", bufs=3) as rl_pool,
            tc.tile_pool(name="outp", bufs=3) as out_pool,
            tc.tile_pool(name="psum", bufs=1, space="PSUM") as psum,
        ):
            # ---- persistent SBUF tensors ----
            xt_q = [
                cpool.tile([128, CK, 512], DT, name=f"xt{q}") for q in range(NJ)
            ]
            wq_t = cpool.tile([128, CK, DL], DT, name="wqt")
            wk_t = cpool.tile([128, CK, DL], DT, name="wkt")
            wv_t = cpool.tile([128, CK, DL], DT, name="wvt")
            wo_t = cpool.tile([128, MD, C], DT, name="wot")
            qt_t = cpool.tile([128, MD, t_len], DT, name="qtt")
            kt_t = cpool.tile([128, MD, t_len], DT, name="ktt")
            v_t = cpool.tile([128, NTT, H_LOC, D + 1], DT, name="vt")
            yt_t = cpool.tile([128, MD, t_len], DT, name="ytt")
            tri_t = cpool.tile([128, 2, 128], DT, name="trit")
            warm_t = cpool.tile([128, 512], DT, name="warmt")

            # ---- input DMAs, ordered by when compute needs them ----
            nc.sync.dma_start(out=tri_t[:, 0, :], in_=tri_d)
            nc.sync.dma_start(out=tri_t[:, 1, :], in_=tri_d)
            nc.sync.dma_start(out=wq_t[:, :, :], in_=wq_d)
            nc.sync.dma_start(out=xt_q[0][:, :, :], in_=x_d[0])
            nc.sync.dma_start(out=wv_t[:, :, :], in_=wv_d)
            nc.sync.dma_start(out=wk_t[:, :, :], in_=wk_d)
            for q in range(1, NJ):
                nc.sync.dma_start(out=xt_q[q][:, :, :], in_=x_d[q])
            nc.sync.dma_start(out=wo_t[:, :, :], in_=wo_d)

            # ones column (index 64) for the l (softmax denominator) rows
            nc.vector.memset(v_t[:, :, :, D : D + 1], 1.0)
            nc.vector.memset(warm_t[:, :], 0.25)
            # partition_broadcast ucode lives in the `proxy` library, not the
            # default-resident `standard` one — load it before any gpsimd op
            nc.gpsimd.load_library(library_config.proxy)

            def warmup(n):
                for _ in range(n):
                    wps = psum.tile([128, 512], F32, name="qkvps", bufs=2)
                    nc.tensor.matmul(
                        wps[:, :],
                        lhsT=warm_t[:, 0:128],
                        rhs=warm_t[:, :],
                        start=True,
                        stop=True,
                    )

            # ---- projection step generators (one yield per matmul) ----
            def qk_steps(m, jlist, parts=("q", "k")):
                for part, w_t, dst_t in (
                    ("q", wq_t, qt_t),
                    ("k", wk_t, kt_t),
                ):
                    if part not in parts:
                        continue
                    for j in jlist:
                        ps = psum.tile([128, 512], F32, name="qkvps", bufs=2)
                        for k in range(CK):
                            nc.tensor.matmul(
                                ps[:, :],
                                lhsT=w_t[:, k, 128 * m : 128 * (m + 1)],
                                rhs=xt_q[j][:, k, :],
                                start=(k == 0),
                                stop=(k == CK - 1),
                            )
                            if k < CK - 1:
                                yield
                        nc.vector.tensor_copy(
                            dst_t[:, m, 512 * j : 512 * (j + 1)], ps[:, :]
                        )
                        yield

            def v_steps(t0, t1):
                for ti in range(t0, t1):
                    q, off = ti // 4, 128 * (ti % 4)
                    ps = psum.tile([128, 512], F32, name="qkvps", bufs=2)
                    for k in range(CK):
                        nc.tensor.matmul(
                            ps[:, :],
                            lhsT=xt_q[q][:, k, off : off + 128],
                            rhs=wv_t[:, k, :],
                            start=(k == 0),
                            stop=(k == CK - 1),
                        )
                        if k < CK - 1:
                            yield
                    nc.vector.tensor_copy(
                        v_t[:, ti, :, 0:D],
                        ps[:, :].rearrange("p (h d) -> p h d", h=H_LOC),
                    )
                    yield

            def proj_steps():
                for ti in range(NTT):
                    tt = slice(128 * ti, 128 * (ti + 1))
                    ot = out_pool.tile([128, C], DT, name="ot")
                    for ci in range(2):
                        cs = slice(512 * ci, 512 * (ci + 1))
                        if ti >= 12:
                            # attention psum is free by now; wider rotation
                            # so the final drain isn't cast-latency bound
                            ps = psum.tile([128, 2, 512], F32, name="sps", bufs=2)
                            ps = ps[:, 0, :]
                        else:
                            ps = psum.tile([128, 512], F32, name="qkvps", bufs=2)
                        for hp in range(MD):
                            nc.tensor.matmul(
                                ps[:, :],
                                lhsT=yt_t[:, hp, tt],
                                rhs=wo_t[:, hp, cs],
                                start=(hp == 0),
                                stop=(hp == MD - 1),
                            )
                            if hp < MD - 1:
                                yield
                        nc.vector.tensor_copy(ot[:, cs], ps[:, :])
                        if ci == 0:
                            yield
                    nc.sync.dma_start(out=out_d[tt, :], in_=ot[:, :])
                    yield

            # ---- filler stream: consumed 2 steps per attention block ----
            def filler_gen():
                yield from qk_steps(0, [1])
                yield from v_steps(4, 8)
                yield "v7"
                yield from qk_steps(0, [2])
                yield from v_steps(8, 12)
                yield "v11"
                yield from qk_steps(0, [3])
                yield from v_steps(12, 16)
                yield "v15"
                for m in range(1, MD):
                    yield from qk_steps(m, range(NJ))
                    yield f"qk{m}"

            fill = {"it": filler_gen(), "seen": set(), "done": False}

            def consume(n):
                if fill["done"]:
                    return
                got = 0
                while got < n:
                    try:
                        item = next(fill["it"])
                    except StopIteration:
                        fill["done"] = True
                        return
                    if isinstance(item, str):
                        fill["seen"].add(item)
                    else:
                        got += 1

            def drain(tag):
                if fill["done"] or tag in fill["seen"]:
                    return
                while True:
                    try:
                        item = next(fill["it"])
                    except StopIteration:
                        fill["done"] = True
                        return
                    if isinstance(item, str):
                        fill["seen"].add(item)
                        if item == tag:
                            return

            proj = {"it": None, "done": True}

            def consume_proj(n):
                if proj["done"]:
                    return
                got = 0
                while got < n:
                    try:
                        next(proj["it"])
                        got += 1
                    except StopIteration:
                        proj["done"] = True
                        return

            # ---- attention stage for head-pair hp ----
            def attn_stage(hp):
                for j in range(NJ):
                    if hp == 0 and j >= 1:
                        drain(f"v{4 * j + 3}")
                    tq0 = 512 * j
                    nblk = 4 * j + 4  # causal: tk blocks 0 .. 4j+3
                    accA = psum.tile([D + 1, 512], F32, name="acc", bufs=2)
                    accB = psum.tile([D + 1, 512], F32, name="acc", bufs=2)
                    pend = []  # software pipeline: AV for block i-1 after S of i

                    def flush_av():
                        for mm in pend:
                            nc.tensor.matmul(**mm)
                        pend.clear()

                    for i in range(nblk):
                        tk = slice(128 * i, 128 * (i + 1))
                        diag = i - 4 * j
                        lo = 128 * diag if diag >= 0 else 0
                        tqs = slice(tq0 + lo, tq0 + 512)
                        sps = psum.tile([128, 2, 512], F32, name="sps", bufs=2)
                        for h2, lohi in ((0, slice(0, 64)), (1, slice(64, 128))):
                            nc.tensor.matmul(
                                sps[:, h2, lo:],
                                lhsT=kt_t[lohi, hp, tk],
                                rhs=qt_t[lohi, hp, tqs],
                                start=True,
                                stop=True,
                            )
                        pt = pt_pool.tile([128, 2, 512], DT, name="pt")
                        nc.scalar.activation(
                            pt[:, :, lo:],
                            sps[:, :, lo:],
                            mybir.ActivationFunctionType.Exp,
                            scale=0.125,
                        )
                        if diag >= 0:  # zero the above-diagonal triangle
                            dg = slice(lo, lo + 128)
                            nc.vector.tensor_mul(
                                pt[:, :, dg], pt[:, :, dg], tri_t[:, :, :]
                            )
                        if hp < MD - 1:
                            consume(2)
                        elif j >= 1:
                            # proj tiles for tq chunk j-1 are fully emitted by
                            # now; consuming earlier would emit proj matmuls
                            # ahead of the yt writes they need (PE deadlock)
                            consume_proj(2)
                        flush_av()
                        for h2, acc in ((0, accA), (1, accB)):
                            pend.append(
                                dict(
                                    out=acc[:, lo:],
                                    lhsT=v_t[:, i, 2 * hp + h2, :],
                                    rhs=pt[:, h2, lo:],
                                    start=(i == 0),
                                    stop=(i == nblk - 1),
                                )
                            )
                    flush_av()

                    # normalization tail. Order matters: the psum-reading
                    # copies (yu, lb) for BOTH heads go first so the acc psum
                    # banks free quickly for the next j / next stage.
                    tq = slice(tq0, tq0 + 512)
                    yus, lbs = [], []
                    for h2, acc in ((0, accA), (1, accB)):
                        yu = yu_pool.tile([64, 512], DT, name="yu")
                        nc.vector.tensor_copy(yu[:, :], acc[0:D, :])
                        lb = rl_pool.tile([1, 512], F32, name="lb")
                        nc.vector.tensor_copy(lb[:, :], acc[D : D + 1, :])
                        yus.append(yu)
                        lbs.append(lb)
                    for h2 in range(2):
                        lbc = rl_pool.tile([64, 512], F32, name="lbc")
                        nc.gpsimd.partition_broadcast(lbc[:, :], lbs[h2][:, :])
                        rli = rl_pool.tile([64, 512], F32, name="rli")
                        nc.vector.reciprocal_approx_fast(rli[:, :], lbc[:, :])
                        nc.vector.tensor_mul(
                            yt_t[64 * h2 : 64 * (h2 + 1), hp, tq],
                            yus[h2][:, :],
                            rli[:, :],
                        )

            # ---- main pipeline ----
            # prologue ordered by DMA arrival: q(j0) [wq,x0], v(0-3) [wv],
            # k(j0) [wk]; the rest of stage-0 q/k flows through the fillers
            warmup(N_WARM)
            for _ in qk_steps(0, [0], parts=("q",)):
                pass
            warmup(N_WARM2)
            for _ in v_steps(0, 4):
                pass
            for _ in qk_steps(0, [0], parts=("k",)):
                pass

            for hp in range(MD):
                if hp >= 1:
                    drain(f"qk{hp}")
                if hp == MD - 1:
                    proj.update(it=proj_steps(), done=False)
                attn_stage(hp)

            consume_proj(10**9)  # finish remaining projection tiles

    nc.compile()
    return nc


def make_host_inputs(x, w_qkv, w_proj, t_len=T):
    """Shard full inputs into the 8 per-core input dicts.

    Everything is pre-arranged into the on-chip SBUF layout
    [partition, chunk, free] (contiguous) so each DMA is one large-run copy.
    """
    NJ = t_len // 512
    tri = np.where(
        np.arange(128)[None, :] >= np.arange(128)[:, None], 1.0, 0.0
    ).astype(BF16)

    def chunked(w, width):  # [C, width] -> [128, C//128, width] contiguous
        return np.ascontiguousarray(
            w.reshape(C // 128, 128, width).transpose(1, 0, 2)
        ).astype(BF16)

    in_maps = []
    for c in range(8):
        b, g = c // 2, c % 2
        xT = np.ascontiguousarray(x[b][:t_len].T)  # [C, T]
        xq = xT.reshape(CK, 128, t_len)
        m = dict(
            wq=chunked(w_qkv[:, 512 * g : 512 * (g + 1)], DL),
            wk=chunked(w_qkv[:, C + 512 * g : C + 512 * (g + 1)], DL),
            wv=chunked(w_qkv[:, 2 * C + 512 * g : 2 * C + 512 * (g + 1)], DL),
            wo=np.ascontiguousarray(
                w_proj[512 * g : 512 * (g + 1), :]
                .reshape(MD := DL // 128, 128, C)
                .transpose(1, 0, 2)
            ).astype(BF16),
            tri=tri,
        )
        for q in range(NJ):
            m[f"x{q}"] = np.ascontiguousarray(
                xq[:, :, 512 * q : 512 * (q + 1)].transpose(1, 0, 2)
            ).astype(BF16)
        in_maps.append(m)
    return in_maps


_CACHE = {}


def _get_program():
    if "nc" not in _CACHE:
        _CACHE["nc"] = build_program()
    return _CACHE["nc"]


def kernel(x, w_qkv, w_proj, _trace=False, _trace_kwargs=None):
    x = np.asarray(x, np.float32)
    w_qkv = np.asarray(w_qkv, np.float32)
    w_proj = np.asarray(w_proj, np.float32)
    nc = _get_program()
    in_maps = make_host_inputs(x, w_qkv, w_proj)
    kw = {}
    if _trace:
        kw = dict(trace=True, **(_trace_kwargs or {}))
    res = run_bass_kernel_spmd(nc, in_maps, core_ids=list(range(8)), **kw)
    out = np.empty((B, T, C), np.float32)
    for b in range(B):
        out[b] = res.results[2 * b]["out"].astype(np.float32) + res.results[
            2 * b + 1
        ]["out"].astype(np.float32)
    if _trace:
        return out, res
    return out
